# revision 16
# baseline (speedup 1.0000x reference)
"""Fused multi-head self-attention (degenerate seq-len-1) + LayerNorm for TRN2.

Math: with sequence length 1, softmax over the single key is exactly 1.0, so
attention output == v.  The whole module collapses to

    out = LayerNorm((x @ W_v.T + b_v) @ W_proj.T + b_proj) * gamma + beta
        = LayerNorm(x @ C.T + bias) * gamma + beta

with C = W_proj @ W_v and bias = W_proj @ b_v + b_proj (both batch-independent,
folded on the host).  The device kernel is a single [1024,4096]x[4096,4096]
matmul per core (batch data-parallel over 8 cores) fused with LayerNorm.

On the host, C's columns (and the bias) are centered so every row of
y = x @ C'.T + b' has exactly zero mean; the device LayerNorm then only needs
the variance (rstd scale), which shortens the per-b-tile epilogue chain.

Schedule notes:
- The kernel head is DMA-supply-bound (~0.32 MB/us): the first accumulation
  groups consume ct chunk 0 + x tiles faster than HBM delivers them.  Dummy
  "warmup" matmuls are interleaved into the first chunk's groups to absorb
  that idle time so the PE's HAM clock gate stays at 2.4 GHz (an idle gap
  >3.4us re-throttles the PE to 1.2 GHz).
- Head DMAs are issued round-robin from all three DMA-capable queues
  (sync/scalar/gpsimd) to avoid the ~0.6us-per-issue serialization.
- Matmul moving dim is 512 (one full PSUM bank); APs are whole-tile, not
  slices -- sliced matmul APs measured +43ns/instruction.
- LayerNorm epilogue is quarter-split across ACT and DVE with output DMAs
  spread over queues, so the per-b-tile chain (~4us) hides under the next
  b-tile's matmul group (~7us).
"""

import os
import sys

import numpy as np

if "/opt/trn_rl_repo" not in sys.path:
    sys.path.insert(0, "/opt/trn_rl_repo")

import ml_dtypes

P = 128              # SBUF partitions
DIM = 4096
B = 8192
NCORES = 8
BL = B // NCORES     # batch rows per core
BT = BL // P         # b tiles per core
KO = DIM // P        # contraction tiles
JC = 512             # moving free-dim chunk (output cols per matmul = 1 bank)
NJC = DIM // JC      # 8 chunks
QW = 1024            # epilogue quarter width
EPS = 1e-5
N_WARM = 24          # PE warmup matmuls before the first real group

_BUILD_CACHE = {}


def _build(apply_bias: bool, apply_affine: bool):
    key = (apply_bias, apply_affine)
    if key in _BUILD_CACHE:
        return _BUILD_CACHE[key]

    import concourse.mybir as mybir
    import concourse.tile as tile
    from concourse import bacc

    bf16 = mybir.dt.bfloat16
    f16 = mybir.dt.float16
    f32 = mybir.dt.float32

    nc = bacc.Bacc("TRN2", target_bir_lowering=False, debug=False,
                   num_devices=NCORES)

    xt_d = nc.declare_dram_parameter("xt", [BT, P, KO, P], bf16, isOutput=False)
    ct_d = nc.declare_dram_parameter("ct", [NJC, P, KO, JC], bf16,
                                     isOutput=False)
    if apply_bias:
        bias_d = nc.declare_dram_parameter("bias", [DIM], f32, isOutput=False)
    if apply_affine:
        gamma_d = nc.declare_dram_parameter("gamma", [DIM], f32, isOutput=False)
        beta_d = nc.declare_dram_parameter("beta", [DIM], f32, isOutput=False)
    # fp16 output (upcast on host): halves the output traffic; 10 mantissa
    # bits is plenty for LayerNorm-scale values.
    out_d = nc.declare_dram_parameter("out", [BT, P, DIM], f16, isOutput=True)

    # With bias the last chunk is evicted to SBUF too (bias-add path); without
    # it the final chunk is normalized straight out of PSUM.
    YCOLS = DIM - JC
    nyc = DIM if apply_bias else YCOLS

    with tile.TileContext(nc) as tc:
        with tc.tile_pool(name="xpool", bufs=1) as xpool, \
             tc.tile_pool(name="wpool", bufs=2) as wpool, \
             tc.tile_pool(name="ypool", bufs=1) as ypool, \
             tc.tile_pool(name="spool", bufs=1) as spool, \
             tc.tile_pool(name="opool", bufs=5) as opool, \
             tc.tile_pool(name="small", bufs=4) as small, \
             tc.tile_pool(name="wscratch", bufs=1) as wscratch, \
             tc.tile_pool(name="ppool", bufs=5, space="PSUM") as ppool, \
             tc.tile_pool(name="pwarm", bufs=1, space="PSUM") as pwarm:

            xt_sb = xpool.tile([P, BT, KO, P], bf16)

            # Warmup scratch: dummy matmuls with no data deps, used both for
            # the initial HAM ramp and to absorb supply-bound idle inside the
            # first chunk's groups (keeping the PE clock at 2.4 GHz).
            warm_sb = wscratch.tile([P, 384], bf16)
            nc.vector.memset(warm_sb, 0.0)
            warm_ps = pwarm.tile([P, 256], f32, name="warm_ps", tag="wps")

            def warm_pack(n):
                for _ in range(n):
                    nc.tensor.matmul(warm_ps, lhsT=warm_sb[:, 0:P],
                                     rhs=warm_sb[:, P:P + 256],
                                     start=True, stop=True)

            warm_pack(N_WARM)

            eps_sb = small.tile([P, 1], f32)
            nc.vector.memset(eps_sb, EPS)

            # ---- head DMA: round-robin across the 3 DMA-capable queues, in
            # first-use order: ct0 ko-split + xt0 split (first accumulation
            # group), then xt1..7 as the b-tile loop consumes them, ct1
            # pieces in the remaining slack.
            issue_engines = [nc.sync, nc.scalar, nc.gpsimd]
            _eng_i = [0]

            def issue(out, in_):
                eng = issue_engines[_eng_i[0] % len(issue_engines)]
                _eng_i[0] += 1
                eng.dma_start(out=out, in_=in_)

            ct_tiles = {}
            ct_tiles[0] = wpool.tile([P, KO, JC], bf16, name="ct_sb", tag="ct")
            ct_tiles[1] = wpool.tile([P, KO, JC], bf16, name="ct_sb", tag="ct")

            # Uniform ~0.5 MiB pieces: the 3 queues drain concurrently at
            # ~1/3 of HBM bandwidth each, so equal piece sizes keep delivery
            # order close to issue order.
            issue(ct_tiles[0][:, 0:4], ct_d[0, :, 0:4])
            issue(xt_sb[:, 0, 0:16], xt_d[0, :, 0:16])
            issue(ct_tiles[0][:, 4:8], ct_d[0, :, 4:8])
            issue(xt_sb[:, 0, 16:32], xt_d[0, :, 16:32])
            for a in range(8, 32, 4):
                issue(ct_tiles[0][:, a:a + 4], ct_d[0, :, a:a + 4])
            for b in range(1, 8):
                issue(xt_sb[:, b, 0:16], xt_d[b, :, 0:16])
                issue(xt_sb[:, b, 16:32], xt_d[b, :, 16:32])
            for a in range(0, 32, 4):
                issue(ct_tiles[1][:, a:a + 4], ct_d[1, :, a:a + 4])

            # y (pre-norm matmul result) stays resident in bf16, flat over
            # columns so the epilogue quarters are simple slices.
            y_sb = ypool.tile([P, BT, nyc], bf16)
            # Per-chunk bn_stats, aggregated per b-tile at the end.
            stats_sb = spool.tile([P, BT, NJC, 6], f32)

            bias_sb = None
            if apply_bias:
                bias_sb = spool.tile([P, DIM], f32)
                nc.sync.dma_start(out=bias_sb,
                                  in_=bias_d.ap().to_broadcast([P, DIM]))

            gamma_sb = beta_sb = None
            if apply_affine:
                gamma_sb = spool.tile([P, DIM], f32)
                nc.sync.dma_start(out=gamma_sb,
                                  in_=gamma_d.ap().to_broadcast([P, DIM]))
                beta_sb = spool.tile([P, DIM], f32)
                nc.sync.dma_start(out=beta_sb,
                                  in_=beta_d.ap().to_broadcast([P, DIM]))

            def affine_fix(o, q):
                if apply_affine:
                    nc.vector.tensor_mul(o, o, gamma_sb[:, q * QW:(q + 1) * QW])
                    nc.vector.tensor_add(o, o, beta_sb[:, q * QW:(q + 1) * QW])

            def apply_piece(bt, rstd, last_ps, lo, hi, engine, dma_eng):
                """Normalize columns [lo:hi) of b-tile bt and DMA them out.
                Columns >= YCOLS come from PSUM (never evicted)."""
                o = opool.tile([P, hi - lo], f16, name="o", tag="o")
                ylo, yhi = lo, min(hi, YCOLS) if not apply_bias else hi
                if yhi > ylo:
                    if engine == "act":
                        nc.scalar.activation(
                            o[:, 0:yhi - lo], y_sb[:, bt, ylo:yhi],
                            mybir.ActivationFunctionType.Identity, scale=rstd)
                    else:
                        nc.vector.tensor_scalar_mul(o[:, 0:yhi - lo],
                                                    y_sb[:, bt, ylo:yhi], rstd)
                if yhi < hi:
                    nc.vector.tensor_scalar_mul(
                        o[:, yhi - lo:hi - lo],
                        last_ps[:, yhi - YCOLS:hi - YCOLS], rstd)
                if apply_affine:
                    nc.vector.tensor_mul(o, o, gamma_sb[:, lo:hi])
                    nc.vector.tensor_add(o, o, beta_sb[:, lo:hi])
                dma_eng.dma_start(out=out_d[bt, :, lo:hi], in_=o)

            def layernorm_apply(bt, last_ps):
                """Scale the b-tile by rstd (rows are exactly zero-mean by
                construction) and stream it out, split across ACT and DVE
                with output DMAs spread over queues.  Interior b-tiles use
                two halves (fewer DMAs/semaphores); the final b-tile uses
                eight pieces on rotating queues to shorten the tail chain."""
                mv = small.tile([P, 2], f32)
                nc.vector.bn_aggr(mv, stats_sb[:, bt, :, :])
                std = small.tile([P, 1], f32)
                nc.scalar.activation(std, mv[:, 1:2],
                                     mybir.ActivationFunctionType.Sqrt,
                                     bias=eps_sb)
                rstd = small.tile([P, 1], f32)
                nc.vector.reciprocal(rstd, std)

                if bt < BT - 1:
                    apply_piece(bt, rstd, last_ps, 0, DIM // 2, "act",
                                nc.gpsimd)
                    apply_piece(bt, rstd, last_ps, DIM // 2, DIM, "dve",
                                nc.sync)
                else:
                    engs = [nc.sync, nc.scalar, nc.gpsimd]
                    for i in range(8):
                        apply_piece(bt, rstd, last_ps, i * JC, (i + 1) * JC,
                                    "act" if i % 2 == 0 else "dve",
                                    engs[i % 3])

            def matmul_group(ct_sb, jc, bt, warm_every=0, warm_n=0):
                off = jc * JC
                ps = ppool.tile([P, JC], f32, name="ps", tag="ps")
                for ko in range(KO):
                    nc.tensor.matmul(
                        ps,
                        lhsT=xt_sb[:, bt, ko, :],
                        rhs=ct_sb[:, ko, :],
                        start=(ko == 0),
                        stop=(ko == KO - 1),
                    )
                    # Absorb DMA-supply idle during the head: dummy matmuls
                    # run while the next ko piece is still in flight.
                    if warm_every and ko % warm_every == warm_every - 1 \
                            and ko != KO - 1:
                        warm_pack(warm_n)
                last_chunk = jc == NJC - 1
                if apply_bias:
                    nc.vector.tensor_add(y_sb[:, bt, off:off + JC], ps,
                                         bias_sb[:, off:off + JC])
                    nc.vector.bn_stats(stats_sb[:, bt, jc, :],
                                       y_sb[:, bt, off:off + JC])
                else:
                    # DVE reads PSUM directly for the LayerNorm statistics;
                    # ACT evicts PSUM (cast to bf16) except for the final
                    # chunk, which the epilogue normalizes straight from PSUM.
                    if not last_chunk:
                        nc.scalar.activation(
                            y_sb[:, bt, off:off + JC], ps,
                            mybir.ActivationFunctionType.Copy)
                    nc.vector.bn_stats(stats_sb[:, bt, jc, :], ps)
                if last_chunk:
                    # Interleaves with the remaining b-tiles' matmuls.
                    layernorm_apply(bt, ps)

            for jc in range(NJC):
                if 1 <= jc < NJC - 1:
                    ct_tiles[jc + 1] = wpool.tile([P, KO, JC], bf16,
                                                  name="ct_sb", tag="ct")
                    nc.scalar.dma_start(out=ct_tiles[jc + 1], in_=ct_d[jc + 1])
                ct_sb = ct_tiles.pop(jc)
                for bt in range(BT):
                    if jc == 0 and 1 <= bt <= 3:
                        warm_pack((14, 0, 0)[bt - 1])
                    if jc == 0 and bt == 0:
                        # group 0 trickles in behind the DMA ramp: ~12 dummy
                        # matmuls per 4-ko piece keep the PE busy/warm.
                        matmul_group(ct_sb, jc, bt, warm_every=4, warm_n=9)
                    else:
                        matmul_group(ct_sb, jc, bt)

    nc.compile()
    _BUILD_CACHE[key] = nc
    return nc


def kernel(x, W_qkv, b_qkv, W_proj, b_proj, gamma, beta):
    from concourse.bass_utils import run_bass_kernel_spmd

    x = np.asarray(x, dtype=np.float32)
    W_qkv = np.asarray(W_qkv, dtype=np.float32)
    b_qkv = np.asarray(b_qkv, dtype=np.float32)
    W_proj = np.asarray(W_proj, dtype=np.float32)
    b_proj = np.asarray(b_proj, dtype=np.float32)
    gamma = np.asarray(gamma, dtype=np.float32)
    beta = np.asarray(beta, dtype=np.float32)

    # Fold the two projections (q/k are dead: seq len 1 => attention == v).
    W_v = W_qkv[2 * DIM:3 * DIM, :]
    C = W_proj @ W_v                          # [j, k]
    bias_total = W_proj @ b_qkv[2 * DIM:] + b_proj

    # Center C's columns and the bias so y's rows are exactly zero-mean:
    # LayerNorm's mean subtraction becomes a no-op and is skipped on device.
    C = C - C.mean(axis=0, keepdims=True)
    bias_total = bias_total - bias_total.mean()

    # C^T tiled for streaming: ct[jc, p, ko, jl] = C[jc*JC+jl, ko*P+p]
    Ct = np.ascontiguousarray(
        C.T.reshape(KO, P, NJC, JC).transpose(2, 1, 0, 3)
    ).astype(ml_dtypes.bfloat16)

    apply_bias = bool(np.any(bias_total))
    apply_affine = not (np.all(gamma == 1.0) and np.all(beta == 0.0))

    nc = _build(apply_bias, apply_affine)

    in_maps = []
    for i in range(NCORES):
        xs = x[i * BL:(i + 1) * BL]           # [BL, DIM]
        # xt[bt, p, ko, b'] = xs[bt*P + b', ko*P + p]
        xt = np.ascontiguousarray(
            xs.T.reshape(KO, P, BT, P).transpose(2, 1, 0, 3)
        ).astype(ml_dtypes.bfloat16)
        im = {"xt": xt, "ct": Ct}
        if apply_bias:
            im["bias"] = bias_total
        if apply_affine:
            im["gamma"] = gamma
            im["beta"] = beta
        in_maps.append(im)

    trace = bool(int(os.environ.get("KERNEL_TRACE", "0")))
    res = run_bass_kernel_spmd(nc, in_maps, core_ids=list(range(NCORES)),
                               trace=trace)
    if trace:
        kernel.last_exec_time_ns = res.exec_time_ns
        kernel.last_results = res

    out = np.concatenate(
        [r["out"].reshape(BL, DIM).astype(np.float32) for r in res.results],
        axis=0,
    )
    return out


# revision 17
# speedup vs baseline: 1.0001x; 1.0001x over previous
"""Fused multi-head self-attention (degenerate seq-len-1) + LayerNorm for TRN2.

Math: with sequence length 1, softmax over the single key is exactly 1.0, so
attention output == v.  The whole module collapses to

    out = LayerNorm((x @ W_v.T + b_v) @ W_proj.T + b_proj) * gamma + beta
        = LayerNorm(x @ C.T + bias) * gamma + beta

with C = W_proj @ W_v and bias = W_proj @ b_v + b_proj (both batch-independent,
folded on the host).  The device kernel is a single [1024,4096]x[4096,4096]
matmul per core (batch data-parallel over 8 cores) fused with LayerNorm.

On the host, C's columns (and the bias) are centered so every row of
y = x @ C'.T + b' has exactly zero mean; the device LayerNorm then only needs
the variance (rstd scale), which shortens the per-b-tile epilogue chain.

Schedule notes:
- The kernel head is DMA-supply-bound (~0.32 MB/us): the first accumulation
  groups consume ct chunk 0 + x tiles faster than HBM delivers them.  Dummy
  "warmup" matmuls are interleaved into the first chunk's groups to absorb
  that idle time so the PE's HAM clock gate stays at 2.4 GHz (an idle gap
  >3.4us re-throttles the PE to 1.2 GHz).
- Head DMAs are issued round-robin from all three DMA-capable queues
  (sync/scalar/gpsimd) to avoid the ~0.6us-per-issue serialization.
- Matmul moving dim is 512 (one full PSUM bank); APs are whole-tile, not
  slices -- sliced matmul APs measured +43ns/instruction.
- LayerNorm epilogue is quarter-split across ACT and DVE with output DMAs
  spread over queues, so the per-b-tile chain (~4us) hides under the next
  b-tile's matmul group (~7us).
"""

import os
import sys

import numpy as np

if "/opt/trn_rl_repo" not in sys.path:
    sys.path.insert(0, "/opt/trn_rl_repo")

import ml_dtypes

P = 128              # SBUF partitions
DIM = 4096
B = 8192
NCORES = 8
BL = B // NCORES     # batch rows per core
BT = BL // P         # b tiles per core
KO = DIM // P        # contraction tiles
JC = 512             # moving free-dim chunk (output cols per matmul = 1 bank)
NJC = DIM // JC      # 8 chunks
QW = 1024            # epilogue quarter width
EPS = 1e-5
N_WARM = 24          # PE warmup matmuls before the first real group

_BUILD_CACHE = {}


def _build(apply_bias: bool, apply_affine: bool):
    key = (apply_bias, apply_affine)
    if key in _BUILD_CACHE:
        return _BUILD_CACHE[key]

    import concourse.mybir as mybir
    import concourse.tile as tile
    from concourse import bacc

    bf16 = mybir.dt.bfloat16
    f16 = mybir.dt.float16
    f32 = mybir.dt.float32

    nc = bacc.Bacc("TRN2", target_bir_lowering=False, debug=False,
                   num_devices=NCORES)

    xt_d = nc.declare_dram_parameter("xt", [BT, P, KO, P], bf16, isOutput=False)
    ct_d = nc.declare_dram_parameter("ct", [NJC, P, KO, JC], bf16,
                                     isOutput=False)
    if apply_bias:
        bias_d = nc.declare_dram_parameter("bias", [DIM], f32, isOutput=False)
    if apply_affine:
        gamma_d = nc.declare_dram_parameter("gamma", [DIM], f32, isOutput=False)
        beta_d = nc.declare_dram_parameter("beta", [DIM], f32, isOutput=False)
    # fp16 output (upcast on host): halves the output traffic; 10 mantissa
    # bits is plenty for LayerNorm-scale values.
    out_d = nc.declare_dram_parameter("out", [BT, P, DIM], f16, isOutput=True)

    # With bias the last chunk is evicted to SBUF too (bias-add path); without
    # it the final chunk is normalized straight out of PSUM.
    YCOLS = DIM - JC
    nyc = DIM if apply_bias else YCOLS

    with tile.TileContext(nc) as tc:
        with tc.tile_pool(name="xpool", bufs=1) as xpool, \
             tc.tile_pool(name="wpool", bufs=2) as wpool, \
             tc.tile_pool(name="ypool", bufs=1) as ypool, \
             tc.tile_pool(name="spool", bufs=1) as spool, \
             tc.tile_pool(name="opool", bufs=5) as opool, \
             tc.tile_pool(name="small", bufs=4) as small, \
             tc.tile_pool(name="wscratch", bufs=1) as wscratch, \
             tc.tile_pool(name="ppool", bufs=5, space="PSUM") as ppool, \
             tc.tile_pool(name="pwarm", bufs=1, space="PSUM") as pwarm:

            xt_sb = xpool.tile([P, BT, KO, P], bf16)

            # Warmup scratch: dummy matmuls with no data deps, used both for
            # the initial HAM ramp and to absorb supply-bound idle inside the
            # first chunk's groups (keeping the PE clock at 2.4 GHz).
            warm_sb = wscratch.tile([P, 384], bf16)
            nc.vector.memset(warm_sb, 0.0)
            warm_ps = pwarm.tile([P, 256], f32, name="warm_ps", tag="wps")

            def warm_pack(n):
                for _ in range(n):
                    nc.tensor.matmul(warm_ps, lhsT=warm_sb[:, 0:P],
                                     rhs=warm_sb[:, P:P + 256],
                                     start=True, stop=True)

            warm_pack(N_WARM)

            eps_sb = small.tile([P, 1], f32)
            nc.vector.memset(eps_sb, EPS)

            # ---- head DMA: round-robin across the 3 DMA-capable queues, in
            # first-use order: ct0 ko-split + xt0 split (first accumulation
            # group), then xt1..7 as the b-tile loop consumes them, ct1
            # pieces in the remaining slack.
            issue_engines = [nc.sync, nc.scalar, nc.gpsimd]
            _eng_i = [0]

            def issue(out, in_):
                eng = issue_engines[_eng_i[0] % len(issue_engines)]
                _eng_i[0] += 1
                eng.dma_start(out=out, in_=in_)

            ct_tiles = {}
            ct_tiles[0] = wpool.tile([P, KO, JC], bf16, name="ct_sb", tag="ct")
            ct_tiles[1] = wpool.tile([P, KO, JC], bf16, name="ct_sb", tag="ct")

            # Uniform ~0.5 MiB pieces: the 3 queues drain concurrently at
            # ~1/3 of HBM bandwidth each, so equal piece sizes keep delivery
            # order close to issue order.
            issue(ct_tiles[0][:, 0:4], ct_d[0, :, 0:4])
            issue(xt_sb[:, 0, 0:16], xt_d[0, :, 0:16])
            issue(ct_tiles[0][:, 4:8], ct_d[0, :, 4:8])
            issue(xt_sb[:, 0, 16:32], xt_d[0, :, 16:32])
            for a in range(8, 32, 4):
                issue(ct_tiles[0][:, a:a + 4], ct_d[0, :, a:a + 4])
            for b in range(1, 8):
                issue(xt_sb[:, b, 0:16], xt_d[b, :, 0:16])
                issue(xt_sb[:, b, 16:32], xt_d[b, :, 16:32])
            for a in range(0, 32, 4):
                issue(ct_tiles[1][:, a:a + 4], ct_d[1, :, a:a + 4])

            # y (pre-norm matmul result) stays resident in bf16, flat over
            # columns so the epilogue quarters are simple slices.
            y_sb = ypool.tile([P, BT, nyc], bf16)
            # Per-chunk bn_stats, aggregated per b-tile at the end.
            stats_sb = spool.tile([P, BT, NJC, 6], f32)

            bias_sb = None
            if apply_bias:
                bias_sb = spool.tile([P, DIM], f32)
                nc.sync.dma_start(out=bias_sb,
                                  in_=bias_d.ap().to_broadcast([P, DIM]))

            gamma_sb = beta_sb = None
            if apply_affine:
                gamma_sb = spool.tile([P, DIM], f32)
                nc.sync.dma_start(out=gamma_sb,
                                  in_=gamma_d.ap().to_broadcast([P, DIM]))
                beta_sb = spool.tile([P, DIM], f32)
                nc.sync.dma_start(out=beta_sb,
                                  in_=beta_d.ap().to_broadcast([P, DIM]))

            def affine_fix(o, q):
                if apply_affine:
                    nc.vector.tensor_mul(o, o, gamma_sb[:, q * QW:(q + 1) * QW])
                    nc.vector.tensor_add(o, o, beta_sb[:, q * QW:(q + 1) * QW])

            def apply_piece(bt, rstd, last_ps, lo, hi, engine, dma_eng):
                """Normalize columns [lo:hi) of b-tile bt and DMA them out.
                Columns >= YCOLS come from PSUM (never evicted)."""
                o = opool.tile([P, hi - lo], f16, name="o", tag="o")
                ylo, yhi = lo, min(hi, YCOLS) if not apply_bias else hi
                if yhi > ylo:
                    if engine == "act":
                        nc.scalar.activation(
                            o[:, 0:yhi - lo], y_sb[:, bt, ylo:yhi],
                            mybir.ActivationFunctionType.Identity, scale=rstd)
                    else:
                        nc.vector.tensor_scalar_mul(o[:, 0:yhi - lo],
                                                    y_sb[:, bt, ylo:yhi], rstd)
                if yhi < hi:
                    nc.vector.tensor_scalar_mul(
                        o[:, yhi - lo:hi - lo],
                        last_ps[:, yhi - YCOLS:hi - YCOLS], rstd)
                if apply_affine:
                    nc.vector.tensor_mul(o, o, gamma_sb[:, lo:hi])
                    nc.vector.tensor_add(o, o, beta_sb[:, lo:hi])
                dma_eng.dma_start(out=out_d[bt, :, lo:hi], in_=o)

            def layernorm_apply(bt, last_ps):
                """Scale the b-tile by rstd (rows are exactly zero-mean by
                construction) and stream it out, split across ACT and DVE
                with output DMAs spread over queues.  Interior b-tiles use
                two halves (fewer DMAs/semaphores); the final b-tile uses
                eight pieces on rotating queues to shorten the tail chain."""
                mv = small.tile([P, 2], f32)
                nc.vector.bn_aggr(mv, stats_sb[:, bt, :, :])
                std = small.tile([P, 1], f32)
                nc.scalar.activation(std, mv[:, 1:2],
                                     mybir.ActivationFunctionType.Sqrt,
                                     bias=eps_sb)
                rstd = small.tile([P, 1], f32)
                nc.vector.reciprocal(rstd, std)

                if bt < BT - 1:
                    apply_piece(bt, rstd, last_ps, 0, DIM // 2, "act",
                                nc.gpsimd)
                    apply_piece(bt, rstd, last_ps, DIM // 2, DIM, "dve",
                                nc.sync)
                else:
                    engs = [nc.sync, nc.scalar, nc.gpsimd]
                    for i in range(8):
                        apply_piece(bt, rstd, last_ps, i * JC, (i + 1) * JC,
                                    "act" if i % 2 == 0 else "dve",
                                    engs[i % 3])

            def matmul_group(ct_sb, jc, bt, warm_every=0, warm_n=0):
                off = jc * JC
                ps = ppool.tile([P, JC], f32, name="ps", tag="ps")
                for ko in range(KO):
                    nc.tensor.matmul(
                        ps,
                        lhsT=xt_sb[:, bt, ko, :],
                        rhs=ct_sb[:, ko, :],
                        start=(ko == 0),
                        stop=(ko == KO - 1),
                    )
                    # Absorb DMA-supply idle during the head: dummy matmuls
                    # run while the next ko piece is still in flight.
                    if warm_every and ko % warm_every == warm_every - 1 \
                            and ko != KO - 1:
                        warm_pack(warm_n)
                last_chunk = jc == NJC - 1
                if apply_bias:
                    nc.vector.tensor_add(y_sb[:, bt, off:off + JC], ps,
                                         bias_sb[:, off:off + JC])
                    nc.vector.bn_stats(stats_sb[:, bt, jc, :],
                                       y_sb[:, bt, off:off + JC])
                else:
                    # DVE reads PSUM directly for the LayerNorm statistics;
                    # ACT evicts PSUM (cast to bf16) except for the final
                    # chunk, which the epilogue normalizes straight from PSUM.
                    if not last_chunk:
                        nc.scalar.activation(
                            y_sb[:, bt, off:off + JC], ps,
                            mybir.ActivationFunctionType.Copy)
                    nc.vector.bn_stats(stats_sb[:, bt, jc, :], ps)
                if last_chunk:
                    # Interleaves with the remaining b-tiles' matmuls.
                    layernorm_apply(bt, ps)

            for jc in range(NJC):
                if 1 <= jc < NJC - 1:
                    ct_tiles[jc + 1] = wpool.tile([P, KO, JC], bf16,
                                                  name="ct_sb", tag="ct")
                    nc.scalar.dma_start(out=ct_tiles[jc + 1], in_=ct_d[jc + 1])
                ct_sb = ct_tiles.pop(jc)
                for bt in range(BT):
                    if jc == 0 and 1 <= bt <= 3:
                        warm_pack((16, 4, 0)[bt - 1])
                    if jc == 0 and bt == 0:
                        # group 0 trickles in behind the DMA ramp: ~12 dummy
                        # matmuls per 4-ko piece keep the PE busy/warm.
                        matmul_group(ct_sb, jc, bt, warm_every=4, warm_n=11)
                    else:
                        matmul_group(ct_sb, jc, bt)

    nc.compile()
    _BUILD_CACHE[key] = nc
    return nc


def kernel(x, W_qkv, b_qkv, W_proj, b_proj, gamma, beta):
    from concourse.bass_utils import run_bass_kernel_spmd

    x = np.asarray(x, dtype=np.float32)
    W_qkv = np.asarray(W_qkv, dtype=np.float32)
    b_qkv = np.asarray(b_qkv, dtype=np.float32)
    W_proj = np.asarray(W_proj, dtype=np.float32)
    b_proj = np.asarray(b_proj, dtype=np.float32)
    gamma = np.asarray(gamma, dtype=np.float32)
    beta = np.asarray(beta, dtype=np.float32)

    # Fold the two projections (q/k are dead: seq len 1 => attention == v).
    W_v = W_qkv[2 * DIM:3 * DIM, :]
    C = W_proj @ W_v                          # [j, k]
    bias_total = W_proj @ b_qkv[2 * DIM:] + b_proj

    # Center C's columns and the bias so y's rows are exactly zero-mean:
    # LayerNorm's mean subtraction becomes a no-op and is skipped on device.
    C = C - C.mean(axis=0, keepdims=True)
    bias_total = bias_total - bias_total.mean()

    # C^T tiled for streaming: ct[jc, p, ko, jl] = C[jc*JC+jl, ko*P+p]
    Ct = np.ascontiguousarray(
        C.T.reshape(KO, P, NJC, JC).transpose(2, 1, 0, 3)
    ).astype(ml_dtypes.bfloat16)

    apply_bias = bool(np.any(bias_total))
    apply_affine = not (np.all(gamma == 1.0) and np.all(beta == 0.0))

    nc = _build(apply_bias, apply_affine)

    in_maps = []
    for i in range(NCORES):
        xs = x[i * BL:(i + 1) * BL]           # [BL, DIM]
        # xt[bt, p, ko, b'] = xs[bt*P + b', ko*P + p]
        xt = np.ascontiguousarray(
            xs.T.reshape(KO, P, BT, P).transpose(2, 1, 0, 3)
        ).astype(ml_dtypes.bfloat16)
        im = {"xt": xt, "ct": Ct}
        if apply_bias:
            im["bias"] = bias_total
        if apply_affine:
            im["gamma"] = gamma
            im["beta"] = beta
        in_maps.append(im)

    trace = bool(int(os.environ.get("KERNEL_TRACE", "0")))
    res = run_bass_kernel_spmd(nc, in_maps, core_ids=list(range(NCORES)),
                               trace=trace)
    if trace:
        kernel.last_exec_time_ns = res.exec_time_ns
        kernel.last_results = res

    out = np.concatenate(
        [r["out"].reshape(BL, DIM).astype(np.float32) for r in res.results],
        axis=0,
    )
    return out


# revision 18
# speedup vs baseline: 1.0086x; 1.0085x over previous
"""Fused multi-head self-attention (degenerate seq-len-1) + LayerNorm for TRN2.

Math: with sequence length 1, softmax over the single key is exactly 1.0, so
attention output == v.  The whole module collapses to

    out = LayerNorm((x @ W_v.T + b_v) @ W_proj.T + b_proj) * gamma + beta
        = LayerNorm(x @ C.T + bias) * gamma + beta

with C = W_proj @ W_v and bias = W_proj @ b_v + b_proj (both batch-independent,
folded on the host).  The device kernel is a single [1024,4096]x[4096,4096]
matmul per core (batch data-parallel over 8 cores) fused with LayerNorm.

On the host, C's columns (and the bias) are centered so every row of
y = x @ C'.T + b' has exactly zero mean; the device LayerNorm then only needs
the variance (rstd scale), which shortens the per-b-tile epilogue chain.

Schedule notes:
- The kernel head is DMA-supply-bound (~0.32 MB/us): the first accumulation
  groups consume ct chunk 0 + x tiles faster than HBM delivers them.  Dummy
  "warmup" matmuls are interleaved into the first chunk's groups to absorb
  that idle time so the PE's HAM clock gate stays at 2.4 GHz (an idle gap
  >3.4us re-throttles the PE to 1.2 GHz).
- Head DMAs are issued round-robin from all three DMA-capable queues
  (sync/scalar/gpsimd) to avoid the ~0.6us-per-issue serialization.
- Matmul moving dim is 512 (one full PSUM bank); APs are whole-tile, not
  slices -- sliced matmul APs measured +43ns/instruction.
- LayerNorm epilogue is quarter-split across ACT and DVE with output DMAs
  spread over queues, so the per-b-tile chain (~4us) hides under the next
  b-tile's matmul group (~7us).
"""

import os
import sys

import numpy as np

if "/opt/trn_rl_repo" not in sys.path:
    sys.path.insert(0, "/opt/trn_rl_repo")

import ml_dtypes

P = 128              # SBUF partitions
DIM = 4096
B = 8192
NCORES = 8
BL = B // NCORES     # batch rows per core
BT = BL // P         # b tiles per core
KO = DIM // P        # contraction tiles
JC = 512             # moving free-dim chunk (output cols per matmul = 1 bank)
NJC = DIM // JC      # 8 chunks
QW = 1024            # epilogue quarter width
EPS = 1e-5
N_WARM = 24          # PE warmup matmuls before the first real group

_BUILD_CACHE = {}


def _build(apply_bias: bool, apply_affine: bool):
    key = (apply_bias, apply_affine)
    if key in _BUILD_CACHE:
        return _BUILD_CACHE[key]

    import concourse.mybir as mybir
    import concourse.tile as tile
    from concourse import bacc

    bf16 = mybir.dt.bfloat16
    f16 = mybir.dt.float16
    f32 = mybir.dt.float32

    nc = bacc.Bacc("TRN2", target_bir_lowering=False, debug=False,
                   num_devices=NCORES)

    xt_d = nc.declare_dram_parameter("xt", [BT, P, KO, P], bf16, isOutput=False)
    ct_d = nc.declare_dram_parameter("ct", [NJC, P, KO, JC], bf16,
                                     isOutput=False)
    if apply_bias:
        bias_d = nc.declare_dram_parameter("bias", [DIM], f32, isOutput=False)
    if apply_affine:
        gamma_d = nc.declare_dram_parameter("gamma", [DIM], f32, isOutput=False)
        beta_d = nc.declare_dram_parameter("beta", [DIM], f32, isOutput=False)
    # fp16 output (upcast on host): halves the output traffic; 10 mantissa
    # bits is plenty for LayerNorm-scale values.
    out_d = nc.declare_dram_parameter("out", [BT, P, DIM], f16, isOutput=True)

    # With bias the last chunk is evicted to SBUF too (bias-add path); without
    # it the final chunk is normalized straight out of PSUM.
    YCOLS = DIM - JC
    nyc = DIM if apply_bias else YCOLS

    with tile.TileContext(nc) as tc:
        with tc.tile_pool(name="xpool", bufs=1) as xpool, \
             tc.tile_pool(name="wpool", bufs=2) as wpool, \
             tc.tile_pool(name="ypool", bufs=1) as ypool, \
             tc.tile_pool(name="spool", bufs=1) as spool, \
             tc.tile_pool(name="opool", bufs=5) as opool, \
             tc.tile_pool(name="small", bufs=4) as small, \
             tc.tile_pool(name="wscratch", bufs=1) as wscratch, \
             tc.tile_pool(name="ppool", bufs=5, space="PSUM") as ppool, \
             tc.tile_pool(name="pwarm", bufs=1, space="PSUM") as pwarm:

            xt_sb = xpool.tile([P, BT, KO, P], bf16)

            # Warmup scratch: dummy matmuls with no data deps, used both for
            # the initial HAM ramp and to absorb supply-bound idle inside the
            # first chunk's groups (keeping the PE clock at 2.4 GHz).
            warm_sb = wscratch.tile([P, 384], bf16)
            nc.vector.memset(warm_sb, 0.0)
            warm_ps = pwarm.tile([P, 256], f32, name="warm_ps", tag="wps")

            def warm_pack(n):
                for _ in range(n):
                    nc.tensor.matmul(warm_ps, lhsT=warm_sb[:, 0:P],
                                     rhs=warm_sb[:, P:P + 256],
                                     start=True, stop=True)

            warm_pack(N_WARM)

            eps_sb = small.tile([P, 1], f32)
            nc.vector.memset(eps_sb, EPS)

            # ---- head DMA: round-robin across the 3 DMA-capable queues, in
            # first-use order: ct0 ko-split + xt0 split (first accumulation
            # group), then xt1..7 as the b-tile loop consumes them, ct1
            # pieces in the remaining slack.
            issue_engines = [nc.sync, nc.scalar, nc.gpsimd]
            _eng_i = [0]

            def issue(out, in_):
                eng = issue_engines[_eng_i[0] % len(issue_engines)]
                _eng_i[0] += 1
                eng.dma_start(out=out, in_=in_)

            ct_tiles = {}
            ct_tiles[0] = wpool.tile([P, KO, JC], bf16, name="ct_sb", tag="ct")
            ct_tiles[1] = wpool.tile([P, KO, JC], bf16, name="ct_sb", tag="ct")

            # Uniform ~0.5 MiB pieces: the 3 queues drain concurrently at
            # ~1/3 of HBM bandwidth each, so equal piece sizes keep delivery
            # order close to issue order.
            issue(ct_tiles[0][:, 0:4], ct_d[0, :, 0:4])
            issue(xt_sb[:, 0, 0:16], xt_d[0, :, 0:16])
            issue(ct_tiles[0][:, 4:8], ct_d[0, :, 4:8])
            issue(xt_sb[:, 0, 16:32], xt_d[0, :, 16:32])
            for a in range(8, 32, 4):
                issue(ct_tiles[0][:, a:a + 4], ct_d[0, :, a:a + 4])
            for b in range(1, 8):
                issue(xt_sb[:, b, 0:16], xt_d[b, :, 0:16])
                issue(xt_sb[:, b, 16:32], xt_d[b, :, 16:32])
            for a in range(0, 32, 4):
                issue(ct_tiles[1][:, a:a + 4], ct_d[1, :, a:a + 4])

            # y (pre-norm matmul result) stays resident in bf16, flat over
            # columns so the epilogue quarters are simple slices.
            y_sb = ypool.tile([P, BT, nyc], bf16)
            # Per-chunk bn_stats, aggregated per b-tile at the end.
            stats_sb = spool.tile([P, BT, NJC, 6], f32)

            bias_sb = None
            if apply_bias:
                bias_sb = spool.tile([P, DIM], f32)
                nc.sync.dma_start(out=bias_sb,
                                  in_=bias_d.ap().to_broadcast([P, DIM]))

            gamma_sb = beta_sb = None
            if apply_affine:
                gamma_sb = spool.tile([P, DIM], f32)
                nc.sync.dma_start(out=gamma_sb,
                                  in_=gamma_d.ap().to_broadcast([P, DIM]))
                beta_sb = spool.tile([P, DIM], f32)
                nc.sync.dma_start(out=beta_sb,
                                  in_=beta_d.ap().to_broadcast([P, DIM]))

            def affine_fix(o, q):
                if apply_affine:
                    nc.vector.tensor_mul(o, o, gamma_sb[:, q * QW:(q + 1) * QW])
                    nc.vector.tensor_add(o, o, beta_sb[:, q * QW:(q + 1) * QW])

            def apply_piece(bt, rstd, last_ps, lo, hi, engine, dma_eng):
                """Normalize columns [lo:hi) of b-tile bt and DMA them out.
                Columns >= YCOLS come from PSUM (never evicted)."""
                o = opool.tile([P, hi - lo], f16, name="o", tag="o")
                ylo, yhi = lo, min(hi, YCOLS) if not apply_bias else hi
                if yhi > ylo:
                    if engine == "act":
                        nc.scalar.activation(
                            o[:, 0:yhi - lo], y_sb[:, bt, ylo:yhi],
                            mybir.ActivationFunctionType.Identity, scale=rstd)
                    else:
                        nc.vector.tensor_scalar_mul(o[:, 0:yhi - lo],
                                                    y_sb[:, bt, ylo:yhi], rstd)
                if yhi < hi:
                    nc.vector.tensor_scalar_mul(
                        o[:, yhi - lo:hi - lo],
                        last_ps[:, yhi - YCOLS:hi - YCOLS], rstd)
                if apply_affine:
                    nc.vector.tensor_mul(o, o, gamma_sb[:, lo:hi])
                    nc.vector.tensor_add(o, o, beta_sb[:, lo:hi])
                dma_eng.dma_start(out=out_d[bt, :, lo:hi], in_=o)

            def layernorm_apply(bt, last_ps):
                """Scale the b-tile by rstd (rows are exactly zero-mean by
                construction) and stream it out, split across ACT and DVE
                with output DMAs spread over queues.  Interior b-tiles use
                two halves (fewer DMAs/semaphores); the final b-tile uses
                eight pieces on rotating queues to shorten the tail chain."""
                mv = small.tile([P, 2], f32)
                nc.vector.bn_aggr(mv, stats_sb[:, bt, :, :])
                std = small.tile([P, 1], f32)
                nc.scalar.activation(std, mv[:, 1:2],
                                     mybir.ActivationFunctionType.Sqrt,
                                     bias=eps_sb)
                rstd = small.tile([P, 1], f32)
                nc.vector.reciprocal(rstd, std)

                if bt < BT - 1:
                    apply_piece(bt, rstd, last_ps, 0, DIM // 2, "act",
                                nc.gpsimd)
                    apply_piece(bt, rstd, last_ps, DIM // 2, DIM, "dve",
                                nc.sync)
                else:
                    engs = [nc.sync, nc.scalar, nc.gpsimd]
                    for i in range(8):
                        apply_piece(bt, rstd, last_ps, i * JC, (i + 1) * JC,
                                    "act" if i % 2 == 0 else "dve",
                                    engs[i % 3])

            def matmul_group(ct_sb, jc, bt, warm_every=0, warm_n=0):
                off = jc * JC
                ps = ppool.tile([P, JC], f32, name="ps", tag="ps")
                for ko in range(KO):
                    nc.tensor.matmul(
                        ps,
                        lhsT=xt_sb[:, bt, ko, :],
                        rhs=ct_sb[:, ko, :],
                        start=(ko == 0),
                        stop=(ko == KO - 1),
                    )
                    # Absorb DMA-supply idle during the head: dummy matmuls
                    # run while the next ko piece is still in flight.
                    if warm_every and ko % warm_every == warm_every - 1 \
                            and ko != KO - 1:
                        warm_pack(warm_n)
                last_chunk = jc == NJC - 1
                if apply_bias:
                    nc.vector.tensor_add(y_sb[:, bt, off:off + JC], ps,
                                         bias_sb[:, off:off + JC])
                    nc.vector.bn_stats(stats_sb[:, bt, jc, :],
                                       y_sb[:, bt, off:off + JC])
                else:
                    # DVE reads PSUM directly for the LayerNorm statistics;
                    # ACT evicts PSUM (cast to bf16) except for the final
                    # chunk, which the epilogue normalizes straight from PSUM.
                    if not last_chunk:
                        nc.scalar.activation(
                            y_sb[:, bt, off:off + JC], ps,
                            mybir.ActivationFunctionType.Copy)
                    nc.vector.bn_stats(stats_sb[:, bt, jc, :], ps)
                if last_chunk:
                    # Interleaves with the remaining b-tiles' matmuls.
                    layernorm_apply(bt, ps)

            for jc in range(NJC):
                if 1 <= jc < NJC - 1:
                    ct_tiles[jc + 1] = wpool.tile([P, KO, JC], bf16,
                                                  name="ct_sb", tag="ct")
                    nc.scalar.dma_start(out=ct_tiles[jc + 1], in_=ct_d[jc + 1])
                ct_sb = ct_tiles.pop(jc)
                for bt in range(BT):
                    if jc == 0 and 1 <= bt <= 3:
                        warm_pack((14, 0, 0)[bt - 1])
                    if jc == 0 and bt == 0:
                        # group 0 trickles in behind the DMA ramp: ~12 dummy
                        # matmuls per 4-ko piece keep the PE busy/warm.
                        matmul_group(ct_sb, jc, bt, warm_every=4, warm_n=9)
                    else:
                        matmul_group(ct_sb, jc, bt)

    nc.compile()
    _BUILD_CACHE[key] = nc
    return nc


def kernel(x, W_qkv, b_qkv, W_proj, b_proj, gamma, beta):
    from concourse.bass_utils import run_bass_kernel_spmd

    x = np.asarray(x, dtype=np.float32)
    W_qkv = np.asarray(W_qkv, dtype=np.float32)
    b_qkv = np.asarray(b_qkv, dtype=np.float32)
    W_proj = np.asarray(W_proj, dtype=np.float32)
    b_proj = np.asarray(b_proj, dtype=np.float32)
    gamma = np.asarray(gamma, dtype=np.float32)
    beta = np.asarray(beta, dtype=np.float32)

    # Fold the two projections (q/k are dead: seq len 1 => attention == v).
    W_v = W_qkv[2 * DIM:3 * DIM, :]
    C = W_proj @ W_v                          # [j, k]
    bias_total = W_proj @ b_qkv[2 * DIM:] + b_proj

    # Center C's columns and the bias so y's rows are exactly zero-mean:
    # LayerNorm's mean subtraction becomes a no-op and is skipped on device.
    C = C - C.mean(axis=0, keepdims=True)
    bias_total = bias_total - bias_total.mean()

    # C^T tiled for streaming: ct[jc, p, ko, jl] = C[jc*JC+jl, ko*P+p]
    Ct = np.ascontiguousarray(
        C.T.reshape(KO, P, NJC, JC).transpose(2, 1, 0, 3)
    ).astype(ml_dtypes.bfloat16)

    apply_bias = bool(np.any(bias_total))
    apply_affine = not (np.all(gamma == 1.0) and np.all(beta == 0.0))

    nc = _build(apply_bias, apply_affine)

    in_maps = []
    for i in range(NCORES):
        xs = x[i * BL:(i + 1) * BL]           # [BL, DIM]
        # xt[bt, p, ko, b'] = xs[bt*P + b', ko*P + p]
        xt = np.ascontiguousarray(
            xs.T.reshape(KO, P, BT, P).transpose(2, 1, 0, 3)
        ).astype(ml_dtypes.bfloat16)
        im = {"xt": xt, "ct": Ct}
        if apply_bias:
            im["bias"] = bias_total
        if apply_affine:
            im["gamma"] = gamma
            im["beta"] = beta
        in_maps.append(im)

    trace = bool(int(os.environ.get("KERNEL_TRACE", "0")))
    res = run_bass_kernel_spmd(nc, in_maps, core_ids=list(range(NCORES)),
                               trace=trace)
    if trace:
        kernel.last_exec_time_ns = res.exec_time_ns
        kernel.last_results = res

    out = np.concatenate(
        [r["out"].reshape(BL, DIM).astype(np.float32) for r in res.results],
        axis=0,
    )
    return out


# revision 19
# speedup vs baseline: 1.0096x; 1.0009x over previous
"""Fused multi-head self-attention (degenerate seq-len-1) + LayerNorm for TRN2.

Math: with sequence length 1, softmax over the single key is exactly 1.0, so
attention output == v.  The whole module collapses to

    out = LayerNorm((x @ W_v.T + b_v) @ W_proj.T + b_proj) * gamma + beta
        = LayerNorm(x @ C.T + bias) * gamma + beta

with C = W_proj @ W_v and bias = W_proj @ b_v + b_proj (both batch-independent,
folded on the host).  The device kernel is a single [1024,4096]x[4096,4096]
matmul per core (batch data-parallel over 8 cores) fused with LayerNorm.

On the host, C's columns (and the bias) are centered so every row of
y = x @ C'.T + b' has exactly zero mean; the device LayerNorm then only needs
the variance (rstd scale), which shortens the per-b-tile epilogue chain.

Schedule notes:
- The kernel head is DMA-supply-bound (~0.32 MB/us): the first accumulation
  groups consume ct chunk 0 + x tiles faster than HBM delivers them.  Dummy
  "warmup" matmuls are interleaved into the first chunk's groups to absorb
  that idle time so the PE's HAM clock gate stays at 2.4 GHz (an idle gap
  >3.4us re-throttles the PE to 1.2 GHz).
- Head DMAs are issued round-robin from all three DMA-capable queues
  (sync/scalar/gpsimd) to avoid the ~0.6us-per-issue serialization.
- Matmul moving dim is 512 (one full PSUM bank); APs are whole-tile, not
  slices -- sliced matmul APs measured +43ns/instruction.
- LayerNorm epilogue is quarter-split across ACT and DVE with output DMAs
  spread over queues, so the per-b-tile chain (~4us) hides under the next
  b-tile's matmul group (~7us).
"""

import os
import sys

import numpy as np

if "/opt/trn_rl_repo" not in sys.path:
    sys.path.insert(0, "/opt/trn_rl_repo")

import ml_dtypes

P = 128              # SBUF partitions
DIM = 4096
B = 8192
NCORES = 8
BL = B // NCORES     # batch rows per core
BT = BL // P         # b tiles per core
KO = DIM // P        # contraction tiles
JC = 512             # moving free-dim chunk (output cols per matmul = 1 bank)
NJC = DIM // JC      # 8 chunks
QW = 1024            # epilogue quarter width
EPS = 1e-5
N_WARM = 24          # PE warmup matmuls before the first real group

_BUILD_CACHE = {}


def _build(apply_bias: bool, apply_affine: bool):
    key = (apply_bias, apply_affine)
    if key in _BUILD_CACHE:
        return _BUILD_CACHE[key]

    import concourse.mybir as mybir
    import concourse.tile as tile
    from concourse import bacc

    bf16 = mybir.dt.bfloat16
    f16 = mybir.dt.float16
    f32 = mybir.dt.float32

    nc = bacc.Bacc("TRN2", target_bir_lowering=False, debug=False,
                   num_devices=NCORES)

    xt_d = nc.declare_dram_parameter("xt", [BT, P, KO, P], bf16, isOutput=False)
    ct_d = nc.declare_dram_parameter("ct", [NJC, P, KO, JC], bf16,
                                     isOutput=False)
    if apply_bias:
        bias_d = nc.declare_dram_parameter("bias", [DIM], f32, isOutput=False)
    if apply_affine:
        gamma_d = nc.declare_dram_parameter("gamma", [DIM], f32, isOutput=False)
        beta_d = nc.declare_dram_parameter("beta", [DIM], f32, isOutput=False)
    # fp16 output (upcast on host): halves the output traffic; 10 mantissa
    # bits is plenty for LayerNorm-scale values.
    out_d = nc.declare_dram_parameter("out", [BT, P, DIM], f16, isOutput=True)

    # With bias the last chunk is evicted to SBUF too (bias-add path); without
    # it the final chunk is normalized straight out of PSUM.
    YCOLS = DIM - JC
    nyc = DIM if apply_bias else YCOLS

    with tile.TileContext(nc) as tc:
        with tc.tile_pool(name="xpool", bufs=1) as xpool, \
             tc.tile_pool(name="wpool", bufs=2) as wpool, \
             tc.tile_pool(name="ypool", bufs=1) as ypool, \
             tc.tile_pool(name="spool", bufs=1) as spool, \
             tc.tile_pool(name="opool", bufs=5) as opool, \
             tc.tile_pool(name="small", bufs=4) as small, \
             tc.tile_pool(name="wscratch", bufs=1) as wscratch, \
             tc.tile_pool(name="ppool", bufs=5, space="PSUM") as ppool, \
             tc.tile_pool(name="pwarm", bufs=1, space="PSUM") as pwarm:

            xt_sb = xpool.tile([P, BT, KO, P], bf16)

            # Warmup scratch: dummy matmuls with no data deps, used both for
            # the initial HAM ramp and to absorb supply-bound idle inside the
            # first chunk's groups (keeping the PE clock at 2.4 GHz).
            warm_sb = wscratch.tile([P, 384], bf16)
            nc.vector.memset(warm_sb, 0.0)
            warm_ps = pwarm.tile([P, 256], f32, name="warm_ps", tag="wps")

            def warm_pack(n):
                for _ in range(n):
                    nc.tensor.matmul(warm_ps, lhsT=warm_sb[:, 0:P],
                                     rhs=warm_sb[:, P:P + 256],
                                     start=True, stop=True)

            warm_pack(N_WARM)

            eps_sb = small.tile([P, 1], f32)
            nc.vector.memset(eps_sb, EPS)

            # ---- head DMA: round-robin across the 3 DMA-capable queues, in
            # first-use order: ct0 ko-split + xt0 split (first accumulation
            # group), then xt1..7 as the b-tile loop consumes them, ct1
            # pieces in the remaining slack.
            issue_engines = [nc.sync, nc.scalar, nc.gpsimd]
            _eng_i = [0]

            def issue(out, in_):
                eng = issue_engines[_eng_i[0] % len(issue_engines)]
                _eng_i[0] += 1
                eng.dma_start(out=out, in_=in_)

            ct_tiles = {}
            ct_tiles[0] = wpool.tile([P, KO, JC], bf16, name="ct_sb", tag="ct")
            ct_tiles[1] = wpool.tile([P, KO, JC], bf16, name="ct_sb", tag="ct")

            # Uniform ~0.5 MiB pieces: the 3 queues drain concurrently at
            # ~1/3 of HBM bandwidth each, so equal piece sizes keep delivery
            # order close to issue order.
            issue(ct_tiles[0][:, 0:4], ct_d[0, :, 0:4])
            issue(xt_sb[:, 0, 0:16], xt_d[0, :, 0:16])
            issue(ct_tiles[0][:, 4:8], ct_d[0, :, 4:8])
            issue(xt_sb[:, 0, 16:32], xt_d[0, :, 16:32])
            for a in range(8, 32, 4):
                issue(ct_tiles[0][:, a:a + 4], ct_d[0, :, a:a + 4])
            for b in range(1, 8):
                issue(xt_sb[:, b, 0:16], xt_d[b, :, 0:16])
                issue(xt_sb[:, b, 16:32], xt_d[b, :, 16:32])
            for a in range(0, 32, 4):
                issue(ct_tiles[1][:, a:a + 4], ct_d[1, :, a:a + 4])

            # y (pre-norm matmul result) stays resident in bf16, flat over
            # columns so the epilogue quarters are simple slices.
            y_sb = ypool.tile([P, BT, nyc], bf16)
            # Per-chunk bn_stats, aggregated per b-tile at the end.
            stats_sb = spool.tile([P, BT, NJC, 6], f32)

            bias_sb = None
            if apply_bias:
                bias_sb = spool.tile([P, DIM], f32)
                nc.sync.dma_start(out=bias_sb,
                                  in_=bias_d.ap().to_broadcast([P, DIM]))

            gamma_sb = beta_sb = None
            if apply_affine:
                gamma_sb = spool.tile([P, DIM], f32)
                nc.sync.dma_start(out=gamma_sb,
                                  in_=gamma_d.ap().to_broadcast([P, DIM]))
                beta_sb = spool.tile([P, DIM], f32)
                nc.sync.dma_start(out=beta_sb,
                                  in_=beta_d.ap().to_broadcast([P, DIM]))

            def affine_fix(o, q):
                if apply_affine:
                    nc.vector.tensor_mul(o, o, gamma_sb[:, q * QW:(q + 1) * QW])
                    nc.vector.tensor_add(o, o, beta_sb[:, q * QW:(q + 1) * QW])

            def apply_piece(bt, rstd, last_ps, lo, hi, engine, dma_eng):
                """Normalize columns [lo:hi) of b-tile bt and DMA them out.
                Columns >= YCOLS come from PSUM (never evicted)."""
                o = opool.tile([P, hi - lo], f16, name="o", tag="o")
                ylo, yhi = lo, min(hi, YCOLS) if not apply_bias else hi
                if yhi > ylo:
                    if engine == "act":
                        nc.scalar.activation(
                            o[:, 0:yhi - lo], y_sb[:, bt, ylo:yhi],
                            mybir.ActivationFunctionType.Identity, scale=rstd)
                    else:
                        nc.vector.tensor_scalar_mul(o[:, 0:yhi - lo],
                                                    y_sb[:, bt, ylo:yhi], rstd)
                if yhi < hi:
                    nc.vector.tensor_scalar_mul(
                        o[:, yhi - lo:hi - lo],
                        last_ps[:, yhi - YCOLS:hi - YCOLS], rstd)
                if apply_affine:
                    nc.vector.tensor_mul(o, o, gamma_sb[:, lo:hi])
                    nc.vector.tensor_add(o, o, beta_sb[:, lo:hi])
                dma_eng.dma_start(out=out_d[bt, :, lo:hi], in_=o)

            def layernorm_apply(bt, last_ps):
                """Scale the b-tile by rstd (rows are exactly zero-mean by
                construction) and stream it out, split across ACT and DVE
                with output DMAs spread over queues.  Interior b-tiles use
                two halves (fewer DMAs/semaphores); the final b-tile uses
                eight pieces on rotating queues to shorten the tail chain."""
                mv = small.tile([P, 2], f32)
                nc.vector.bn_aggr(mv, stats_sb[:, bt, :, :])
                std = small.tile([P, 1], f32)
                nc.scalar.activation(std, mv[:, 1:2],
                                     mybir.ActivationFunctionType.Sqrt,
                                     bias=eps_sb)
                rstd = small.tile([P, 1], f32)
                nc.vector.reciprocal(rstd, std)

                if bt < BT - 1:
                    # Keep gpsimd's queue empty in the last chunk so its slow
                    # end-of-kernel DRAIN overlaps the epilogue.
                    apply_piece(bt, rstd, last_ps, 0, DIM // 2, "act",
                                nc.scalar)
                    apply_piece(bt, rstd, last_ps, DIM // 2, DIM, "dve",
                                nc.sync)
                else:
                    engs = [nc.sync, nc.scalar, nc.gpsimd]
                    for i in range(8):
                        apply_piece(bt, rstd, last_ps, i * JC, (i + 1) * JC,
                                    "act" if i % 2 == 0 else "dve",
                                    engs[i % 3])

            def matmul_group(ct_sb, jc, bt, warm_every=0, warm_n=0):
                off = jc * JC
                ps = ppool.tile([P, JC], f32, name="ps", tag="ps")
                for ko in range(KO):
                    nc.tensor.matmul(
                        ps,
                        lhsT=xt_sb[:, bt, ko, :],
                        rhs=ct_sb[:, ko, :],
                        start=(ko == 0),
                        stop=(ko == KO - 1),
                    )
                    # Absorb DMA-supply idle during the head: dummy matmuls
                    # run while the next ko piece is still in flight.
                    if warm_every and ko % warm_every == warm_every - 1 \
                            and ko != KO - 1:
                        warm_pack(warm_n)
                last_chunk = jc == NJC - 1
                if apply_bias:
                    nc.vector.tensor_add(y_sb[:, bt, off:off + JC], ps,
                                         bias_sb[:, off:off + JC])
                    nc.vector.bn_stats(stats_sb[:, bt, jc, :],
                                       y_sb[:, bt, off:off + JC])
                else:
                    # DVE reads PSUM directly for the LayerNorm statistics;
                    # ACT evicts PSUM (cast to bf16) except for the final
                    # chunk, which the epilogue normalizes straight from PSUM.
                    if not last_chunk:
                        nc.scalar.activation(
                            y_sb[:, bt, off:off + JC], ps,
                            mybir.ActivationFunctionType.Copy)
                    nc.vector.bn_stats(stats_sb[:, bt, jc, :], ps)
                if last_chunk:
                    # Interleaves with the remaining b-tiles' matmuls.
                    layernorm_apply(bt, ps)

            for jc in range(NJC):
                if 1 <= jc < NJC - 1:
                    ct_tiles[jc + 1] = wpool.tile([P, KO, JC], bf16,
                                                  name="ct_sb", tag="ct")
                    nc.scalar.dma_start(out=ct_tiles[jc + 1], in_=ct_d[jc + 1])
                ct_sb = ct_tiles.pop(jc)
                for bt in range(BT):
                    if jc == 0 and 1 <= bt <= 3:
                        warm_pack((14, 0, 0)[bt - 1])
                    if jc == 0 and bt == 0:
                        # group 0 trickles in behind the DMA ramp: ~12 dummy
                        # matmuls per 4-ko piece keep the PE busy/warm.
                        matmul_group(ct_sb, jc, bt, warm_every=4, warm_n=9)
                    else:
                        matmul_group(ct_sb, jc, bt)

    nc.compile()
    _BUILD_CACHE[key] = nc
    return nc


def kernel(x, W_qkv, b_qkv, W_proj, b_proj, gamma, beta):
    from concourse.bass_utils import run_bass_kernel_spmd

    x = np.asarray(x, dtype=np.float32)
    W_qkv = np.asarray(W_qkv, dtype=np.float32)
    b_qkv = np.asarray(b_qkv, dtype=np.float32)
    W_proj = np.asarray(W_proj, dtype=np.float32)
    b_proj = np.asarray(b_proj, dtype=np.float32)
    gamma = np.asarray(gamma, dtype=np.float32)
    beta = np.asarray(beta, dtype=np.float32)

    # Fold the two projections (q/k are dead: seq len 1 => attention == v).
    W_v = W_qkv[2 * DIM:3 * DIM, :]
    C = W_proj @ W_v                          # [j, k]
    bias_total = W_proj @ b_qkv[2 * DIM:] + b_proj

    # Center C's columns and the bias so y's rows are exactly zero-mean:
    # LayerNorm's mean subtraction becomes a no-op and is skipped on device.
    C = C - C.mean(axis=0, keepdims=True)
    bias_total = bias_total - bias_total.mean()

    # C^T tiled for streaming: ct[jc, p, ko, jl] = C[jc*JC+jl, ko*P+p]
    Ct = np.ascontiguousarray(
        C.T.reshape(KO, P, NJC, JC).transpose(2, 1, 0, 3)
    ).astype(ml_dtypes.bfloat16)

    apply_bias = bool(np.any(bias_total))
    apply_affine = not (np.all(gamma == 1.0) and np.all(beta == 0.0))

    nc = _build(apply_bias, apply_affine)

    in_maps = []
    for i in range(NCORES):
        xs = x[i * BL:(i + 1) * BL]           # [BL, DIM]
        # xt[bt, p, ko, b'] = xs[bt*P + b', ko*P + p]
        xt = np.ascontiguousarray(
            xs.T.reshape(KO, P, BT, P).transpose(2, 1, 0, 3)
        ).astype(ml_dtypes.bfloat16)
        im = {"xt": xt, "ct": Ct}
        if apply_bias:
            im["bias"] = bias_total
        if apply_affine:
            im["gamma"] = gamma
            im["beta"] = beta
        in_maps.append(im)

    trace = bool(int(os.environ.get("KERNEL_TRACE", "0")))
    res = run_bass_kernel_spmd(nc, in_maps, core_ids=list(range(NCORES)),
                               trace=trace)
    if trace:
        kernel.last_exec_time_ns = res.exec_time_ns
        kernel.last_results = res

    out = np.concatenate(
        [r["out"].reshape(BL, DIM).astype(np.float32) for r in res.results],
        axis=0,
    )
    return out
